# revision 8
# baseline (speedup 1.0000x reference)
"""Causal linear attention (elu+1 feature map) on 8 trn2 NeuronCores.

Sharding: core = 4*b + g  (b = batch 0..1, g = head-group 0..3, 4 heads each).
Per core: qkv projection for its 4 heads (w_attn column shard), chunked causal
linear attention (DxD state recurrence in PSUM), row-sharded output projection
giving a partial (T, C) output. Host sums the 4 head-group partials per batch.

Layouts on chip (per core):
  xT   (C=1024, T=2048) fp32r   - x[b] transposed on host
  q,k  feature-major (64, 4 heads, T) fp16 after phi=elu+1
  v    token-major (128, 4, 65) fp16 per t-tile, col 64 = ones (for denominator)
  attention chunk L=128: AT = K Q^T (s,t) -> mask -> Y^T = V'^T AT + S'^T Q
  state S' = [S | z] (64, 65) accumulated in PSUM across chunks
  proj: out(t, c) = lhsT(Y^T tile).T @ w_proj_shard
"""

import numpy as np

import concourse.bass as bass
import concourse.mybir as mybir
import concourse.tile as tile
from concourse import bacc
from concourse.bass_utils import run_bass_kernel_spmd

F32 = mybir.dt.float32
F32R = mybir.dt.float32r
F16 = mybir.dt.float16
AF = mybir.ActivationFunctionType
ALU = mybir.AluOpType

B, T, C = 2, 2048, 1024
H, D = 16, 64
NCORES = 8
HL = 4  # heads per core
FQK = HL * D  # 256 local features for each of q, k, v
L = 128  # attention chunk length
NCH = T // L  # 16 chunks
KT = C // 128  # 8 contraction tiles


def build_nc(dbg=False):
    nc = bacc.Bacc("TRN2", target_bir_lowering=False, debug=False, num_devices=NCORES)

    xt_d = nc.dram_tensor("xt", (C, T), F32R, kind="ExternalInput")
    ws_d = nc.dram_tensor("ws", (C, 3 * FQK), F32R, kind="ExternalInput")
    wp_d = nc.dram_tensor("wp", (FQK, C), F16, kind="ExternalInput")
    mask_d = nc.dram_tensor("mask", (L, L), F32, kind="ExternalInput")
    id_d = nc.dram_tensor("ident", (D, D), F16, kind="ExternalInput")
    out_d = nc.dram_tensor("out", (T, C), F32, kind="ExternalOutput")
    if dbg:
        dq_d = nc.dram_tensor("dbg_q", (64, HL, T), F16, kind="ExternalOutput")
        dk_d = nc.dram_tensor("dbg_k", (64, HL, T), F16, kind="ExternalOutput")
        dv_d = nc.dram_tensor("dbg_v", (NCH, 128, HL, D + 1), F16, kind="ExternalOutput")
        dyt_d = nc.dram_tensor("dbg_yt", (2, 128, T), F16, kind="ExternalOutput")
        dden_d = nc.dram_tensor("dbg_den", (97, T), F32, kind="ExternalOutput")
        dyd_d = nc.dram_tensor("dbg_yd", (2, 128, T), F16, kind="ExternalOutput")

    xt_ap = xt_d.ap().rearrange("(k p) t -> k p t", p=128)  # (8, 128, 2048)
    ws_ap = ws_d.ap().rearrange("(k p) f -> k p f", p=128)  # (8, 128, 768)
    wp_ap = wp_d.ap().rearrange("(k p) c -> k p c", p=128)  # (2, 128, 1024)

    with tile.TileContext(nc) as tc:
        with (
            tc.tile_pool(name="consts", bufs=1) as consts,
            tc.tile_pool(name="wsp", bufs=1) as wsp,
            tc.tile_pool(name="qk", bufs=1) as qkp,
            tc.tile_pool(name="vp", bufs=1) as vp,
            tc.tile_pool(name="ytp", bufs=1) as ytp,
            tc.tile_pool(name="divp", bufs=1) as divp,
            tc.tile_pool(name="epi", bufs=3) as epi,
            tc.tile_pool(name="attsb", bufs=3) as attsb,
            tc.tile_pool(name="rbp", bufs=2) as rbp,
            tc.tile_pool(name="outp", bufs=3) as outp,
        ):
            # ---- constants ----
            mask_sb = consts.tile([L, L], F32)
            nc.sync.dma_start(out=mask_sb[:], in_=mask_d.ap()[:])
            id_sb = consts.tile([D, D], F16)
            nc.sync.dma_start(out=id_sb[:], in_=id_d.ap()[:])

            # ---- weights ----
            ws_sb = wsp.tile([128, KT, 3 * FQK], F32R)
            for kk in range(KT):
                nc.sync.dma_start(out=ws_sb[:, kk, :], in_=ws_ap[kk])
            wp_sb = wsp.tile([128, 2, C], F16)
            for kk in range(2):
                nc.sync.dma_start(out=wp_sb[:, kk, :], in_=wp_ap[kk])

            # ---- persistent activations ----
            q_sb = qkp.tile([64, HL, T], F16)
            k_sb = qkp.tile([64, HL, T], F16)
            v_tiles = [vp.tile([128, HL, D + 1], F16, tag=f"v{tt}", name=f"v{tt}") for tt in range(NCH)]
            ytpair = [ytp.tile([128, T], F16, tag=f"ytp{p_}", name=f"ytp{p_}") for p_ in range(2)]
            den4 = divp.tile([97, T], F32)  # heads at partitions 0/32/64/96
            ydiv = [divp.tile([128, T], F16, tag=f"yd{hp}", name=f"yd{hp}") for hp in range(2)]

            with tc.tile_pool(name="xtp", bufs=1) as xtp:
                xt_sb = xtp.tile([128, KT, T], F32R)
                for kk in range(KT):
                    nc.sync.dma_start(out=xt_sb[:, kk, :], in_=xt_ap[kk])

                with tc.tile_pool(name="qkps", bufs=3, space="PSUM") as qkps, \
                     tc.tile_pool(name="vps", bufs=2, space="PSUM") as vps:
                    # ---- qkv: q and k, feature-major (f, t) ----
                    # fo 0..1 -> q heads (0,1),(2,3); fo 2..3 -> k heads
                    for fo in range(4):
                        dst = q_sb if fo < 2 else k_sb
                        for tb in range(4):
                            ps = qkps.tile([128, 512], F32, tag="qk")
                            for kk in range(KT):
                                nc.tensor.matmul(
                                    ps[:],
                                    ws_sb[:, kk, fo * 128 : (fo + 1) * 128],
                                    xt_sb[:, kk, tb * 512 : (tb + 1) * 512],
                                    start=(kk == 0),
                                    stop=(kk == KT - 1),
                                )
                            # phi = min(exp(x),1) + relu(x), output fp16
                            for half in range(2):
                                hh = (fo % 2) * 2 + half
                                rows = slice(half * 64, half * 64 + 64)
                                tsl = slice(tb * 512, (tb + 1) * 512)
                                e_t = epi.tile([64, 512], F16, tag="e")
                                nc.scalar.activation(
                                    out=e_t[:], in_=ps[rows, :], func=AF.Exp
                                )
                                m_t = epi.tile([64, 512], F16, tag="m")
                                nc.gpsimd.tensor_scalar_min(m_t[:], e_t[:], 1.0)
                                # (psum max 0) + m  on DVE
                                nc.vector.scalar_tensor_tensor(
                                    out=dst[:, hh, tsl],
                                    in0=ps[rows, :],
                                    scalar=0.0,
                                    in1=m_t[:],
                                    op0=ALU.max,
                                    op1=ALU.add,
                                )

                    # ---- qkv: v token-major ----
                    for tt in range(NCH):
                        psv = vps.tile([128, FQK], F32, tag="v")
                        for kk in range(KT):
                            nc.tensor.matmul(
                                psv[:],
                                xt_sb[:, kk, tt * 128 : (tt + 1) * 128],
                                ws_sb[:, kk, 2 * FQK : 3 * FQK],
                                start=(kk == 0),
                                stop=(kk == KT - 1),
                            )
                        vt = v_tiles[tt]
                        nc.vector.memset(vt[:, :, D : D + 1], 1.0)
                        nc.scalar.copy(
                            out=vt[:, :, 0:D],
                            in_=psv[:].rearrange("p (h d) -> p h d", d=D),
                        )

            # ---- attention ----
            with tc.tile_pool(name="atps", bufs=1, space="PSUM") as atps, \
                 tc.tile_pool(name="trps", bufs=1, space="PSUM") as trps, \
                 tc.tile_pool(name="ytps", bufs=2, space="PSUM") as ytps, \
                 tc.tile_pool(name="sps", bufs=1, space="PSUM") as sps:
                s_tiles = [
                    sps.tile([64, D + 1], F32, tag=f"s{h}", name=f"s{h}")
                    for h in range(HL)
                ]
                s_of = lambda h: s_tiles[h][:]

                for i in range(NCH):
                    tsl = slice(i * L, (i + 1) * L)
                    for h in range(HL):
                        qd = q_sb[:, h, tsl]
                        kd = k_sb[:, h, tsl]
                        vh = v_tiles[i][:, h, :]

                        at = atps.tile([L, L], F32, tag="at")
                        nc.tensor.matmul(at[:], kd, qd, start=True, stop=True)
                        atm = attsb.tile([L, L], F16, tag="atm")
                        nc.vector.tensor_mul(atm[:], at[:], mask_sb[:])

                        if i < NCH - 1:
                            ktr = trps.tile([L, D], F16, tag="ktr")
                            nc.tensor.transpose(ktr[:], kd, id_sb[:])
                            ktok = attsb.tile([L, D], F16, tag="ktok")
                            nc.scalar.copy(out=ktok[:], in_=ktr[:])

                        if i > 0:
                            ssb = attsb.tile([64, D + 1], F16, tag="ssb")
                            nc.scalar.copy(out=ssb[:], in_=s_of(h))

                        yt = ytps.tile([D + 1, L], F32, tag="yt")
                        nc.tensor.matmul(
                            yt[:], vh, atm[:], start=True, stop=(i == 0)
                        )
                        if i > 0:
                            nc.tensor.matmul(
                                yt[:], ssb[:], qd, start=False, stop=True
                            )

                        if i < NCH - 1:
                            nc.tensor.matmul(
                                s_of(h),
                                ktok[:],
                                vh,
                                start=(i == 0),
                                stop=(i == NCH - 2),
                            )

                        hp, hr = h // 2, (h % 2) * 64
                        nc.scalar.copy(
                            out=ytpair[hp][hr : hr + 64, tsl], in_=yt[0:D, :]
                        )
                        nc.vector.tensor_copy(
                            out=den4[32 * h : 32 * h + 1, tsl],
                            in_=yt[D : D + 1, :],
                        )

                        # division tail per head, after its last chunk
                        if i == NCH - 1:
                            rt = rbp.tile([1, T], F16, tag="rt")
                            with nc.allow_low_precision(reason="recip to fp16"):
                                nc.vector.reciprocal(
                                    out=rt[:],
                                    in_=den4[32 * h : 32 * h + 1, :],
                                )
                            rb = rbp.tile([128, T], F16, tag="rb")
                            nc.gpsimd.partition_broadcast(rb[:], rt[:])
                            nc.vector.tensor_mul(
                                ydiv[hp][hr : hr + 64, :],
                                ytpair[hp][hr : hr + 64, :],
                                rb[hr : hr + 64, :],
                            )

            if dbg:
                nc.sync.dma_start(out=dq_d.ap()[:], in_=q_sb[:])
                nc.sync.dma_start(out=dk_d.ap()[:], in_=k_sb[:])
                for tt in range(NCH):
                    nc.sync.dma_start(out=dv_d.ap()[tt], in_=v_tiles[tt][:])
                for p_ in range(2):
                    nc.sync.dma_start(out=dyt_d.ap()[p_], in_=ytpair[p_][:])
                nc.sync.dma_start(out=dden_d.ap()[:], in_=den4[:])
                for hp in range(2):
                    nc.sync.dma_start(out=dyd_d.ap()[hp], in_=ydiv[hp][:])

            # ---- output projection ----
            with tc.tile_pool(name="pps", bufs=3, space="PSUM") as pps:
                for tt in range(NCH):
                    tsl = slice(tt * 128, (tt + 1) * 128)
                    po = [None, None]
                    for cb in range(2):
                        ps = pps.tile([128, 512], F32, tag="po")
                        for hp in range(2):
                            nc.tensor.matmul(
                                ps[:],
                                ydiv[hp][:, tsl],
                                wp_sb[:, hp, cb * 512 : (cb + 1) * 512],
                                start=(hp == 0),
                                stop=(hp == 1),
                            )
                        po[cb] = ps
                    os_t = outp.tile([128, C], F32, tag="os")
                    for cb in range(2):
                        nc.scalar.copy(
                            out=os_t[:, cb * 512 : (cb + 1) * 512], in_=po[cb][:]
                        )
                    nc.sync.dma_start(out=out_d.ap()[tsl, :], in_=os_t[:])

    nc.compile()
    return nc


_NC = None


def _get_nc():
    global _NC
    if _NC is None:
        _NC = build_nc()
    return _NC


def make_in_maps(x, w_attn, w_proj):
    mask = np.triu(np.ones((L, L), dtype=np.float32))
    ident = np.eye(D, dtype=np.float16)
    in_maps = []
    for core in range(NCORES):
        b, g = core // HL, core % HL
        cols = slice(g * FQK, (g + 1) * FQK)
        xt = np.ascontiguousarray(x[b].T)
        ws = np.ascontiguousarray(
            np.concatenate(
                [w_attn[:, 0 * C :][:, cols], w_attn[:, C : 2 * C][:, cols],
                 w_attn[:, 2 * C :][:, cols]],
                axis=1,
            )
        )
        wp = np.ascontiguousarray(w_proj[g * FQK : (g + 1) * FQK, :]).astype(
            np.float16
        )
        in_maps.append(dict(xt=xt, ws=ws, wp=wp, mask=mask, ident=ident))
    return in_maps


def kernel(x, w_attn, w_proj):
    x = np.asarray(x)
    w_attn = np.asarray(w_attn)
    w_proj = np.asarray(w_proj)
    nc = _get_nc()
    res = run_bass_kernel_spmd(
        nc, make_in_maps(x, w_attn, w_proj), core_ids=list(range(NCORES))
    )
    out = np.zeros((B, T, C), dtype=np.float32)
    for core in range(NCORES):
        out[core // HL] += res.results[core]["out"]
    return out


if __name__ == "__main__":
    rng = np.random.default_rng(0)
    x = rng.standard_normal((B, T, C)).astype(np.float32)
    wa = (rng.standard_normal((C, 3 * C)) * 0.02).astype(np.float32)
    wp = (rng.standard_normal((C, C)) * 0.02).astype(np.float32)
    o = kernel(x, wa, wp)
    print("out", o.shape, o.dtype, float(np.abs(o).max()))


# revision 9
# speedup vs baseline: 2.1381x; 2.1381x over previous
"""Causal linear attention (elu+1 feature map) on 8 trn2 NeuronCores.

Sharding: core = 4*b + g  (b = batch 0..1, g = head-group 0..3, 4 heads each).
Per core: qkv projection for its 4 heads (w_attn column shard), chunked causal
linear attention (DxD state recurrence in PSUM), row-sharded output projection
giving a partial (T, C) output. Host sums the 4 head-group partials per batch.

Layouts on chip (per core):
  xT   (C=1024, T=2048) fp32r   - x[b] transposed on host
  q,k  feature-major (64, 4 heads, T) fp16 after phi=elu+1
  v    token-major (128, 4, 65) fp16 per t-tile, col 64 = ones (for denominator)
  attention chunk L=128: AT = K Q^T (s,t) -> mask -> Y^T = V'^T AT + S'^T Q
  state S' = [S | z] (64, 65) accumulated in PSUM across chunks
  proj: out(t, c) = lhsT(Y^T tile).T @ w_proj_shard
"""

import numpy as np

import concourse.bass as bass
import concourse.mybir as mybir
import concourse.tile as tile
from concourse import bacc
from concourse.bass_utils import run_bass_kernel_spmd

F32 = mybir.dt.float32
F32R = mybir.dt.float32r
F16 = mybir.dt.float16
AF = mybir.ActivationFunctionType
ALU = mybir.AluOpType

B, T, C = 2, 2048, 1024
H, D = 16, 64
NCORES = 8
HL = 4  # heads per core
FQK = HL * D  # 256 local features for each of q, k, v
L = 128  # attention chunk length
NCH = T // L  # 16 chunks
KT = C // 128  # 8 contraction tiles


def build_nc(dbg=False):
    nc = bacc.Bacc("TRN2", target_bir_lowering=False, debug=False, num_devices=NCORES)

    xt_d = nc.dram_tensor("xt", (C, T), F32R, kind="ExternalInput")
    ws_d = nc.dram_tensor("ws", (C, 3 * FQK), F32R, kind="ExternalInput")
    wp_d = nc.dram_tensor("wp", (FQK, C), F16, kind="ExternalInput")
    mask_d = nc.dram_tensor("mask", (L, L), F32, kind="ExternalInput")
    id_d = nc.dram_tensor("ident", (D, D), F16, kind="ExternalInput")
    out_d = nc.dram_tensor("out", (T, C), F32, kind="ExternalOutput")
    if dbg:
        dq_d = nc.dram_tensor("dbg_q", (64, HL, T), F16, kind="ExternalOutput")
        dk_d = nc.dram_tensor("dbg_k", (64, HL, T), F16, kind="ExternalOutput")
        dv_d = nc.dram_tensor("dbg_v", (NCH, 128, HL, D + 1), F16, kind="ExternalOutput")
        dyt_d = nc.dram_tensor("dbg_yt", (2, 128, T), F16, kind="ExternalOutput")
        dden_d = nc.dram_tensor("dbg_den", (97, T), F32, kind="ExternalOutput")
        dyd_d = nc.dram_tensor("dbg_yd", (2, 128, T), F16, kind="ExternalOutput")

    xt_ap = xt_d.ap().rearrange("(k p) t -> k p t", p=128)  # (8, 128, 2048)
    ws_ap = ws_d.ap().rearrange("(k p) f -> k p f", p=128)  # (8, 128, 768)
    wp_ap = wp_d.ap().rearrange("(k p) c -> k p c", p=128)  # (2, 128, 1024)

    with tile.TileContext(nc) as tc:
        with (
            tc.tile_pool(name="consts", bufs=1) as consts,
            tc.tile_pool(name="wsp", bufs=1) as wsp,
            tc.tile_pool(name="qk", bufs=1) as qkp,
            tc.tile_pool(name="vp", bufs=1) as vp,
            tc.tile_pool(name="ytp", bufs=1) as ytp,
            tc.tile_pool(name="divp", bufs=1) as divp,
            tc.tile_pool(name="epi", bufs=3) as epi,
            tc.tile_pool(name="attsb", bufs=3) as attsb,
            tc.tile_pool(name="rbp", bufs=2) as rbp,
            tc.tile_pool(name="outp", bufs=3) as outp,
        ):
            # ---- constants ----
            mask_sb = consts.tile([L, L], F32)
            nc.sync.dma_start(out=mask_sb[:], in_=mask_d.ap()[:])
            id_sb = consts.tile([D, D], F16)
            nc.sync.dma_start(out=id_sb[:], in_=id_d.ap()[:])

            # ---- weights ----
            ws_sb = wsp.tile([128, KT, 3 * FQK], F32R)
            for kk in range(KT):
                nc.sync.dma_start(out=ws_sb[:, kk, :], in_=ws_ap[kk])
            wp_sb = wsp.tile([128, 2, C], F16)
            for kk in range(2):
                nc.sync.dma_start(out=wp_sb[:, kk, :], in_=wp_ap[kk])

            # ---- persistent activations ----
            q_sb = qkp.tile([64, HL, T], F16)
            k_sb = qkp.tile([64, HL, T], F16)
            v_tiles = [vp.tile([128, HL, D + 1], F16, tag=f"v{tt}", name=f"v{tt}") for tt in range(NCH)]
            ytpair = [ytp.tile([128, T], F16, tag=f"ytp{p_}", name=f"ytp{p_}") for p_ in range(2)]
            den4 = divp.tile([97, T], F32)  # heads at partitions 0/32/64/96
            r97 = divp.tile([97, T], F32)
            ydiv = [divp.tile([128, T], F16, tag=f"yd{hp}", name=f"yd{hp}") for hp in range(2)]

            with tc.tile_pool(name="xtp", bufs=1) as xtp:
                xt_sb = xtp.tile([128, KT, T], F32R)
                for kk in range(KT):
                    nc.sync.dma_start(out=xt_sb[:, kk, :], in_=xt_ap[kk])

                with tc.tile_pool(name="qkps", bufs=3, space="PSUM") as qkps, \
                     tc.tile_pool(name="vps", bufs=2, space="PSUM") as vps:
                    # ---- qkv: q and k, feature-major (f, t) ----
                    # fo 0..1 -> q heads (0,1),(2,3); fo 2..3 -> k heads
                    for fo in range(4):
                        dst = q_sb if fo < 2 else k_sb
                        for tb in range(4):
                            ps = qkps.tile([128, 512], F32, tag="qk")
                            for kk in range(KT):
                                nc.tensor.matmul(
                                    ps[:],
                                    ws_sb[:, kk, fo * 128 : (fo + 1) * 128],
                                    xt_sb[:, kk, tb * 512 : (tb + 1) * 512],
                                    start=(kk == 0),
                                    stop=(kk == KT - 1),
                                )
                            # phi = min(exp(x),1) + relu(x), output fp16
                            for half in range(2):
                                hh = (fo % 2) * 2 + half
                                rows = slice(half * 64, half * 64 + 64)
                                tsl = slice(tb * 512, (tb + 1) * 512)
                                e_t = epi.tile([64, 512], F16, tag="e")
                                nc.scalar.activation(
                                    out=e_t[:], in_=ps[rows, :], func=AF.Exp
                                )
                                m_t = epi.tile([64, 512], F16, tag="m")
                                nc.vector.tensor_scalar_min(m_t[:], e_t[:], 1.0)
                                # (psum max 0) + m  on DVE
                                nc.vector.scalar_tensor_tensor(
                                    out=dst[:, hh, tsl],
                                    in0=ps[rows, :],
                                    scalar=0.0,
                                    in1=m_t[:],
                                    op0=ALU.max,
                                    op1=ALU.add,
                                )

                    # ---- qkv: v token-major ----
                    for tt in range(NCH):
                        psv = vps.tile([128, FQK], F32, tag="v")
                        for kk in range(KT):
                            nc.tensor.matmul(
                                psv[:],
                                xt_sb[:, kk, tt * 128 : (tt + 1) * 128],
                                ws_sb[:, kk, 2 * FQK : 3 * FQK],
                                start=(kk == 0),
                                stop=(kk == KT - 1),
                            )
                        vt = v_tiles[tt]
                        nc.vector.memset(vt[:, :, D : D + 1], 1.0)
                        nc.scalar.copy(
                            out=vt[:, :, 0:D],
                            in_=psv[:].rearrange("p (h d) -> p h d", d=D),
                        )

            # ---- attention ----
            with tc.tile_pool(name="atps", bufs=1, space="PSUM") as atps, \
                 tc.tile_pool(name="trps", bufs=1, space="PSUM") as trps, \
                 tc.tile_pool(name="ytps", bufs=2, space="PSUM") as ytps, \
                 tc.tile_pool(name="sps", bufs=1, space="PSUM") as sps:
                s_tiles = [
                    sps.tile([64, D + 1], F32, tag=f"s{h}", name=f"s{h}")
                    for h in range(HL)
                ]
                s_of = lambda h: s_tiles[h][:]

                for i in range(NCH):
                    tsl = slice(i * L, (i + 1) * L)
                    for h in range(HL):
                        qd = q_sb[:, h, tsl]
                        kd = k_sb[:, h, tsl]
                        vh = v_tiles[i][:, h, :]

                        at = atps.tile([L, L], F32, tag="at")
                        nc.tensor.matmul(at[:], kd, qd, start=True, stop=True)
                        atm = attsb.tile([L, L], F16, tag="atm")
                        nc.vector.tensor_mul(atm[:], at[:], mask_sb[:])

                        if i < NCH - 1:
                            ktr = trps.tile([L, D], F16, tag="ktr")
                            nc.tensor.transpose(ktr[:], kd, id_sb[:])
                            ktok = attsb.tile([L, D], F16, tag="ktok")
                            nc.scalar.copy(out=ktok[:], in_=ktr[:])

                        if i > 0:
                            ssb = attsb.tile([64, D + 1], F16, tag="ssb")
                            nc.scalar.copy(out=ssb[:], in_=s_of(h))

                        yt = ytps.tile([D + 1, L], F32, tag="yt")
                        nc.tensor.matmul(
                            yt[:], vh, atm[:], start=True, stop=(i == 0)
                        )
                        if i > 0:
                            nc.tensor.matmul(
                                yt[:], ssb[:], qd, start=False, stop=True
                            )

                        if i < NCH - 1:
                            nc.tensor.matmul(
                                s_of(h),
                                ktok[:],
                                vh,
                                start=(i == 0),
                                stop=(i == NCH - 2),
                            )

                        hp, hr = h // 2, (h % 2) * 64
                        nc.scalar.copy(
                            out=ytpair[hp][hr : hr + 64, tsl], in_=yt[0:D, :]
                        )
                        nc.vector.tensor_copy(
                            out=den4[32 * h : 32 * h + 1, tsl],
                            in_=yt[D : D + 1, :],
                        )


            if dbg:
                nc.sync.dma_start(out=dq_d.ap()[:], in_=q_sb[:])
                nc.sync.dma_start(out=dk_d.ap()[:], in_=k_sb[:])
                for tt in range(NCH):
                    nc.sync.dma_start(out=dv_d.ap()[tt], in_=v_tiles[tt][:])
                for p_ in range(2):
                    nc.sync.dma_start(out=dyt_d.ap()[p_], in_=ytpair[p_][:])
                nc.sync.dma_start(out=dden_d.ap()[:], in_=den4[:])
                for hp in range(2):
                    nc.sync.dma_start(out=dyd_d.ap()[hp], in_=ydiv[hp][:])

            # ---- division ----
            nc.vector.reciprocal_approx_fast(out=r97[:], in_=den4[:])
            for h in range(HL):
                hp, hr = h // 2, (h % 2) * 64
                rt = rbp.tile([1, T], F16, tag="rt")
                with nc.allow_low_precision(reason="r to fp16"):
                    nc.vector.tensor_copy(
                        out=rt[:], in_=r97[32 * h : 32 * h + 1, :]
                    )
                rb = rbp.tile([128, T], F16, tag="rb")
                nc.gpsimd.partition_broadcast(rb[:], rt[:])
                nc.vector.tensor_mul(
                    ydiv[hp][hr : hr + 64, :],
                    ytpair[hp][hr : hr + 64, :],
                    rb[hr : hr + 64, :],
                )

            # ---- output projection ----
            with tc.tile_pool(name="pps", bufs=3, space="PSUM") as pps:
                for tt in range(NCH):
                    tsl = slice(tt * 128, (tt + 1) * 128)
                    po = [None, None]
                    for cb in range(2):
                        ps = pps.tile([128, 512], F32, tag="po")
                        for hp in range(2):
                            nc.tensor.matmul(
                                ps[:],
                                ydiv[hp][:, tsl],
                                wp_sb[:, hp, cb * 512 : (cb + 1) * 512],
                                start=(hp == 0),
                                stop=(hp == 1),
                            )
                        po[cb] = ps
                    os_t = outp.tile([128, C], F32, tag="os")
                    for cb in range(2):
                        nc.scalar.copy(
                            out=os_t[:, cb * 512 : (cb + 1) * 512], in_=po[cb][:]
                        )
                    nc.sync.dma_start(out=out_d.ap()[tsl, :], in_=os_t[:])

    nc.compile()
    return nc


_NC = None


def _get_nc():
    global _NC
    if _NC is None:
        _NC = build_nc()
    return _NC


def make_in_maps(x, w_attn, w_proj):
    mask = np.triu(np.ones((L, L), dtype=np.float32))
    ident = np.eye(D, dtype=np.float16)
    in_maps = []
    for core in range(NCORES):
        b, g = core // HL, core % HL
        cols = slice(g * FQK, (g + 1) * FQK)
        xt = np.ascontiguousarray(x[b].T)
        ws = np.ascontiguousarray(
            np.concatenate(
                [w_attn[:, 0 * C :][:, cols], w_attn[:, C : 2 * C][:, cols],
                 w_attn[:, 2 * C :][:, cols]],
                axis=1,
            )
        )
        wp = np.ascontiguousarray(w_proj[g * FQK : (g + 1) * FQK, :]).astype(
            np.float16
        )
        in_maps.append(dict(xt=xt, ws=ws, wp=wp, mask=mask, ident=ident))
    return in_maps


def kernel(x, w_attn, w_proj):
    x = np.asarray(x)
    w_attn = np.asarray(w_attn)
    w_proj = np.asarray(w_proj)
    nc = _get_nc()
    res = run_bass_kernel_spmd(
        nc, make_in_maps(x, w_attn, w_proj), core_ids=list(range(NCORES))
    )
    out = np.zeros((B, T, C), dtype=np.float32)
    for core in range(NCORES):
        out[core // HL] += res.results[core]["out"]
    return out


if __name__ == "__main__":
    rng = np.random.default_rng(0)
    x = rng.standard_normal((B, T, C)).astype(np.float32)
    wa = (rng.standard_normal((C, 3 * C)) * 0.02).astype(np.float32)
    wp = (rng.standard_normal((C, C)) * 0.02).astype(np.float32)
    o = kernel(x, wa, wp)
    print("out", o.shape, o.dtype, float(np.abs(o).max()))
